# revision 2
# baseline (speedup 1.0000x reference)
"""Trainium2 Bass kernel: 4096x4096 fp32 image, 9x9 valid cross-correlation + bias.

Strategy
--------
Row-shard across the 8 NeuronCores (511 output rows each plus an 8-row halo,
per the sharding hint); the 9x9 kernel (as a banded-Toeplitz stationary) and
bias are replicated. No collectives: the host hands each core its row stripe
and concatenates the row stripes of the output.

The on-device structure is driven by two empirical laws of this execution
path (measured via repeat-delta microbenchmarks):

  1. every unique instruction in the stream costs ~30-60us to process, and
     instructions with register-offset (dynamic) access patterns cost ~20us
     per execution, while STATIC-AP instructions re-executed by a For_i
     hardware loop cost only ~2us -> keep the stream small and make the hot
     instructions static by moving DATA through fixed addresses;
  2. DMA costs ~0.3-1us per descriptor, so descriptors should be full 16KB
     image rows (row-sharding) rather than 2KB column-stripe rows.

Per core: 5 static DMAs load 5 overlapping 128-row blocks (block b holds
local input rows [120b, 120b+128) x 4096 cols, 16KB descriptors) into one
SBUF tile. A For_i hardware loop over the 5 row blocks then:
  - copies block b into a fixed staging tile (DVE, the only dynamic-src op),
  - runs 8 column chunks x 9 accumulating matmuls with fully STATIC access
    patterns (stationary = 128x120 band of the kernel column dj; moving =
    [128, 511] slice of the stage at static offset c*511+dj; psum = bank c),
  - drains each psum bank to an output row tile fused with the bias add
    (DVE tensor_scalar_add, static),
  - stores [120, 4088] with one DMA (16KB descriptors, dynamic dst row).
Output rows 480..599 of blocks covering past row 511 are computed from
zero-padded input rows and dropped on the host.
"""

import numpy as np

H, W = 4096, 4096
KH, KW = 9, 9
NCORES = 8
OH, OW = H - KH + 1, W - KW + 1  # 4088, 4088
RPC = OH // NCORES  # 511 output rows per core
IN_ROWS = RPC + KH - 1  # 519 input rows per core (8-row halo)
MB = 120  # output rows per block
NB = 5  # row blocks per core (covers 600 rows; 511..599 garbage)
NCH = 8  # column chunks of 511
PAD_ROWS = (NB - 1) * MB + 128  # 608 padded input rows per core


def _build_nc(repeat=1):
    import concourse.bacc as bacc
    import concourse.mybir as mybir
    import concourse.tile as tile
    from concourse import bass
    from concourse.bass import ds

    F32 = mybir.dt.float32
    nc = bacc.Bacc("TRN2", target_bir_lowering=False, debug=False)
    Xs = nc.dram_tensor("Xs", [PAD_ROWS, W], F32, kind="ExternalInput")
    Bm = nc.dram_tensor("Bm", [128, KW * MB], F32, kind="ExternalInput")
    Bc = nc.dram_tensor("Bc", [1, 1], F32, kind="ExternalInput")
    O = nc.dram_tensor("O", [NB * MB, OW], F32, kind="ExternalOutput")

    with tile.TileContext(nc) as tc:
        with (
            tc.tile_pool(name="const", bufs=1) as cpool,
            tc.tile_pool(name="op", bufs=2) as op,
            tc.tile_pool(name="pp", bufs=8, space="PSUM") as pp,
        ):
            b_sb = cpool.tile([128, KW * MB], F32)
            nc.sync.dma_start(b_sb[:], Bm[:])
            bias_col = cpool.tile([128, 1], F32)
            nc.sync.dma_start(
                bias_col[:], bass.AP(tensor=Bc, offset=0, ap=[[0, 128], [1, 1]])
            )
            xt = cpool.tile([128, NB * W], F32)
            stage = cpool.tile([128, W], F32)

            for _ in range(repeat):
                for b in range(NB):
                    nc.sync.dma_start(
                        xt[:, b * W : (b + 1) * W], Xs[b * MB : b * MB + 128, :]
                    )
                with tc.For_i(0, NB) as b:
                    ot = op.tile([128, OW], F32, tag="ot")
                    nc.vector.tensor_copy(stage[:, :], xt[:, ds(b * W, W)])
                    for c in range(NCH):
                        ps = pp.tile([128, RPC], F32, tag="ps")
                        for dj in range(KW):
                            nc.tensor.matmul(
                                ps[:MB, :RPC],
                                b_sb[:128, dj * MB : (dj + 1) * MB],
                                stage[:128, c * RPC + dj : c * RPC + dj + RPC],
                                start=(dj == 0),
                                stop=(dj == KW - 1),
                            )
                        nc.vector.tensor_scalar_add(
                            ot[:MB, c * RPC : (c + 1) * RPC],
                            ps[:MB, :RPC],
                            bias_col[:MB, 0:1],
                        )
                    nc.sync.dma_start(O[ds(b * MB, MB), :], ot[:MB, :])

    nc.compile()
    return nc


def _host_inputs(X, kern, bias):
    """Per-core input maps: row-sharded X with halo + replicated band/bias."""
    X = np.asarray(X, dtype=np.float32)
    kern = np.asarray(kern, dtype=np.float32)
    bias = np.asarray(bias, dtype=np.float32)

    Bm = np.zeros((128, KW * MB), np.float32)
    m = np.arange(MB)
    for dj in range(KW):
        for d in range(KH):
            Bm[m + d, dj * MB + m] = kern[d, dj]
    Bc = np.array([[bias[0]]], np.float32)

    maps = []
    for c in range(NCORES):
        Xs = np.zeros((PAD_ROWS, W), np.float32)
        Xs[:IN_ROWS] = X[RPC * c : RPC * c + IN_ROWS]
        maps.append({"Xs": Xs, "Bm": Bm, "Bc": Bc})
    return maps


_NC_CACHE = {}


def _get_nc(repeat=1):
    if repeat not in _NC_CACHE:
        _NC_CACHE[repeat] = _build_nc(repeat)
    return _NC_CACHE[repeat]


def kernel(X, kernel, bias):
    from concourse.bass_utils import run_bass_kernel_spmd

    nc = _get_nc()
    in_maps = _host_inputs(X, kernel, bias)
    res = run_bass_kernel_spmd(nc, in_maps, core_ids=list(range(NCORES)))
    out = np.empty((OH, OW), np.float32)
    for c in range(NCORES):
        out[RPC * c : RPC * (c + 1), :] = res.results[c]["O"][:RPC]
    return out
